# revision 3
# baseline (speedup 1.0000x reference)
"""Trainium2 Bass kernel for: out = relu(einsum('bcs,cs->bs', x, w) + bias).

Full shapes: x [32, 2048, 4096] f32, w [2048, 4096] f32, bias [4096] f32.
Sharding: the s-axis (4096) is split across 8 cores (512 each). Each core
reads its x slice (128 MiB) and w/bias slice (4 MiB) once — the minimum
possible HBM traffic — and produces out[:, s_slice]. Gather = concat.

Per-core dataflow (partitions = 16-channel group, free = (cb, s)):
  DMA   x[b] slice  -> SBUF [128, 8192]             (4 MiB per batch)
  DVE   prod = xb * w   (fp32 mul, bf16 output tile)
  PE    ones-matmul per 512-col block, accumulating the 128-partition
        reduction of each [128, 512] product block into PSUM [1, 512];
        the bias row is folded in as a K=1 fp32 matmul that opens the
        accumulation group.
  ACT   relu during PSUM -> SBUF copy into out row b
  DMA   out [32, 512] -> DRAM

Layout: channels are mapped p-outer — partition p holds channels
[16p, 16p+16) — so each partition's DMA line is 32 KiB contiguous in
DRAM (the cb-outer layout gives 16x 2 KiB strided lines, which measured
345 GB/s busy-rate vs the 358 GB/s HBM wire). The channel permutation
is applied consistently to x and w, and the reduction is permutation-
invariant, so the result is unchanged.

Precision: the x*w products are rounded to bf16 before the PE reduction
(the reduction itself accumulates in fp32 PSUM). bf16 moving operands
stream at 1 col/cycle vs 4 for fp32, which keeps PE (~3.5 us/batch) and
DVE (~8.6 us/batch, no fold offload needed) both well under the 11.8 us
DMA period — so the end-of-stream drain is just the last chunk's chain
instead of a ~25 us engine backlog. Product rounding error accumulates
as ~2^-9 relative l2 on the output (gate is 2e-2).
"""

import numpy as np

B, C, S_FULL = 32, 2048, 4096
N_CORES = 8
S = S_FULL // N_CORES          # 512 s-values per core
P = 128                        # SBUF partitions
CB = C // P                    # 16 channel blocks (512-col matmul blocks)
F = CB * S                     # 8192 free elems per partition

_nc_cache = {}


def _build():
    import concourse.bacc as bacc
    import concourse.mybir as mybir
    import concourse.tile as tile

    f32 = mybir.dt.float32
    bf16 = mybir.dt.bfloat16
    nc = bacc.Bacc(
        "TRN2",
        target_bir_lowering=False,
        debug=False,
        enable_asserts=False,
        num_devices=N_CORES,
    )

    # p-outer channel layout: x[b, p, q*S + s] = x_orig[b, 16p + q, s].
    # Each partition's per-batch line is one 32 KiB contiguous DRAM run.
    x = nc.dram_tensor("xs", [B, P, F], f32, kind="ExternalInput").ap()
    w = nc.dram_tensor("ws", [P, F], f32, kind="ExternalInput").ap()
    bias = nc.dram_tensor("bs", [1, S], f32, kind="ExternalInput").ap()
    out = nc.dram_tensor("out", [B, S], f32, kind="ExternalOutput").ap()

    with tile.TileContext(nc) as tc:
        with (
            tc.tile_pool(name="const", bufs=1) as cpool,
            tc.tile_pool(name="xp", bufs=3) as xpool,
            tc.tile_pool(name="pp", bufs=2) as ppool,
            tc.tile_pool(name="ps", bufs=4, space="PSUM") as pspool,
            tc.tile_pool(name="op", bufs=1) as opool,
        ):
            # w leads the Sync ring ahead of the x stream: a second
            # concurrent HWDGE stream measures ~8% slower per packet,
            # which costs more than the serial weight load.
            w_sb = cpool.tile([P, F], f32)
            nc.sync.dma_start(w_sb[:], w[:])

            # lhsT of the reduction matmuls, bf16 to match the product
            # tiles (memset can't write bf16; round via DVE copy).
            ones_f32 = cpool.tile([P, 1], f32)
            nc.vector.memset(ones_f32[:], 1.0)
            ones = cpool.tile([P, 1], bf16)
            nc.vector.tensor_copy(ones[:], ones_f32[:])

            # scalar ring: keeps this 2 KiB transfer (and its trigger) out
            # of the w -> x0 handoff on the sync ring
            bias_sb = cpool.tile([1, S], f32)
            nc.scalar.dma_start(bias_sb[:], bias[:])

            # Single-partition output staging: compute engines may only
            # address APs with a 32-aligned base partition, so out rows
            # live along the free axis at partition 0, drained in halves.
            HALF = B // 2
            out_sb = opool.tile([1, HALF * S], f32)

            for b in range(B):
                # One 4 MiB load per batch minimizes per-trigger overhead.
                # DVE can only start a chunk's mul after the whole chunk
                # lands, so whole-batch chunks leave DVE a full 8.6 us mul
                # behind the stream at the end; the last two batches load
                # in halves then eighths so the post-stream drain is one
                # small chunk's chain, not a batch's.
                xb = xpool.tile([P, F], f32, tag="xb")
                prod = ppool.tile([P, F], bf16, tag="prod")
                nchunk = 8 if b == B - 1 else (2 if b == B - 2 else 1)
                CH = F // nchunk
                nblk = CB // nchunk
                ps = pspool.tile([1, S], f32)
                # bias fold-in: K=1 fp32 matmul opens the accumulation
                # group (512 cols, exact, negligible PE time)
                nc.tensor.matmul(
                    ps[:], ones_f32[0:1, 0:1], bias_sb[:], start=True, stop=False
                )
                for h in range(nchunk):
                    r0 = h * CH
                    r1 = (h + 1) * CH
                    nc.sync.dma_start(xb[:, r0:r1], x[b, :, r0:r1])
                    nc.vector.tensor_mul(
                        prod[:, r0:r1], xb[:, r0:r1], w_sb[:, r0:r1]
                    )
                    for i in range(nblk):
                        cb = h * nblk + i
                        rhs = prod[:, cb * S : (cb + 1) * S]
                        last = (h == nchunk - 1) and (i == nblk - 1)
                        nc.tensor.matmul(
                            ps[:], ones[:], rhs, start=False, stop=last
                        )

                nc.scalar.activation(
                    out_sb[0:1, (b % HALF) * S : (b % HALF + 1) * S],
                    ps[:],
                    mybir.ActivationFunctionType.Relu,
                )
                if b == HALF - 1:
                    # Scalar ring: on the sync ring this drain's wait-on-ACT
                    # would block later x triggers (FIFO per engine).
                    nc.scalar.dma_start(
                        out[0:HALF].unsqueeze(0),
                        out_sb[:].rearrange("p (b s) -> p b s", b=HALF),
                    )
                if b == B - 2:
                    # Rows 16..30 drain mid-stream too, so the post-ACT
                    # serial tail only carries row 31's 2 KiB.
                    nc.scalar.dma_start(
                        out[HALF : B - 1].unsqueeze(0),
                        out_sb[:, 0 : (HALF - 1) * S].rearrange(
                            "p (b s) -> p b s", b=HALF - 1
                        ),
                    )

            nc.sync.dma_start(
                out[B - 1 :].unsqueeze(0),
                out_sb[:, (HALF - 1) * S :].rearrange("p (b s) -> p b s", b=1),
            )

    nc.compile()
    return nc


def _get_nc():
    if "nc" not in _nc_cache:
        _nc_cache["nc"] = _build()
    return _nc_cache["nc"]


def _shard_inputs(x, weights, bias):
    x = np.asarray(x)
    weights = np.asarray(weights)
    bias = np.asarray(bias)
    in_maps = []
    for i in range(N_CORES):
        sl = slice(i * S, (i + 1) * S)
        # The [B, 2048, S] shard reshaped to [B, 128, 8192] realizes the
        # p-outer layout: (c, s) -> (c // 16, (c % 16) * 512 + s).
        in_maps.append(
            {
                "xs": np.ascontiguousarray(
                    x[:, :, sl], dtype=np.float32
                ).reshape(B, P, F),
                "ws": np.ascontiguousarray(
                    weights[:, sl], dtype=np.float32
                ).reshape(P, F),
                "bs": np.ascontiguousarray(
                    bias[sl].reshape(1, S), dtype=np.float32
                ),
            }
        )
    return in_maps


def _run(inputs, trace=False, trace_cores=None):
    from concourse import bass_utils

    nc = _get_nc()
    in_maps = _shard_inputs(inputs["x"], inputs["weights"], inputs["bias"])
    res = bass_utils.run_bass_kernel_spmd(
        nc,
        in_maps,
        core_ids=list(range(N_CORES)),
        trace=trace,
        trace_cores=trace_cores,
    )
    out = np.concatenate([r["out"] for r in res.results], axis=1)
    return out, res


def kernel(x, weights, bias):
    out, _ = _run({"x": x, "weights": weights, "bias": bias})
    return out


# revision 5
# speedup vs baseline: 1.1966x; 1.1966x over previous
"""Trainium2 Bass kernel for: out = relu(einsum('bcs,cs->bs', x, w) + bias).

Full shapes: x [32, 2048, 4096] f32, w [2048, 4096] f32, bias [4096] f32.
Sharding: the s-axis (4096) is split across 8 cores (512 each). Each core
reads its x slice (128 MiB) and w/bias slice (4 MiB) once — the minimum
possible HBM traffic — and produces out[:, s_slice]. Gather = concat.

Per-core dataflow (partitions = 16-channel group, free = (cb, s)):
  DMA   x[b] slice  -> SBUF [128, 8192]             (4 MiB per batch)
  DVE   prod = xb * w   (fp32 mul, bf16 output tile)
  PE    ones-matmul per 512-col block, accumulating the 128-partition
        reduction of each [128, 512] product block into PSUM [1, 512];
        the bias row is folded in as a K=1 fp32 matmul that opens the
        accumulation group.
  ACT   relu during PSUM -> SBUF copy into out row b
  DMA   out [32, 512] -> DRAM

Layout: channels are mapped p-outer — partition p holds channels
[16p, 16p+16) — so each partition's DMA line is 32 KiB contiguous in
DRAM (the cb-outer layout gives 16x 2 KiB strided lines, which measured
345 GB/s busy-rate vs the 358 GB/s HBM wire). The channel permutation
is applied consistently to x and w, and the reduction is permutation-
invariant, so the result is unchanged.

Precision: the x*w products are rounded to bf16 before the PE reduction
(the reduction itself accumulates in fp32 PSUM). bf16 moving operands
stream at 1 col/cycle vs 4 for fp32, which keeps PE (~3.5 us/batch) and
DVE (~8.6 us/batch, no fold offload needed) both well under the 11.8 us
DMA period — so the end-of-stream drain is just the last chunk's chain
instead of a ~25 us engine backlog. Product rounding error accumulates
as ~2^-9 relative l2 on the output (gate is 2e-2).
"""

import numpy as np

B, C, S_FULL = 32, 2048, 4096
N_CORES = 8
S = S_FULL // N_CORES          # 512 s-values per core
P = 128                        # SBUF partitions
CB = C // P                    # 16 channel blocks (512-col matmul blocks)
F = CB * S                     # 8192 free elems per partition

_nc_cache = {}


def _build():
    import concourse.bacc as bacc
    import concourse.mybir as mybir
    import concourse.tile as tile

    f32 = mybir.dt.float32
    bf16 = mybir.dt.bfloat16
    nc = bacc.Bacc(
        "TRN2",
        target_bir_lowering=False,
        debug=False,
        enable_asserts=False,
        num_devices=N_CORES,
    )

    # p-outer channel layout: x[b, p, q*S + s] = x_orig[b, 16p + q, s].
    # Each partition's per-batch line is one 32 KiB contiguous DRAM run.
    x = nc.dram_tensor("xs", [B, P, F], f32, kind="ExternalInput").ap()
    w = nc.dram_tensor("ws", [P, F], f32, kind="ExternalInput").ap()
    bias = nc.dram_tensor("bs", [1, S], f32, kind="ExternalInput").ap()
    out = nc.dram_tensor("out", [B, S], f32, kind="ExternalOutput").ap()

    with tile.TileContext(nc) as tc:
        with (
            tc.tile_pool(name="const", bufs=1) as cpool,
            tc.tile_pool(name="xp", bufs=3) as xpool,
            tc.tile_pool(name="pp", bufs=2) as ppool,
            tc.tile_pool(name="ps", bufs=4, space="PSUM") as pspool,
            tc.tile_pool(name="op", bufs=1) as opool,
        ):
            # w leads the Sync ring ahead of the x stream: a second
            # concurrent HWDGE stream measures ~8% slower per packet,
            # which costs more than the serial weight load.
            w_sb = cpool.tile([P, F], f32)
            nc.sync.dma_start(w_sb[:], w[:])

            # lhsT of the reduction matmuls, bf16 to match the product
            # tiles (memset can't write bf16; round via DVE copy).
            ones_f32 = cpool.tile([P, 1], f32)
            nc.vector.memset(ones_f32[:], 1.0)
            ones = cpool.tile([P, 1], bf16)
            nc.vector.tensor_copy(ones[:], ones_f32[:])

            # scalar ring: keeps this 2 KiB transfer (and its trigger) out
            # of the w -> x0 handoff on the sync ring
            bias_sb = cpool.tile([1, S], f32)
            nc.scalar.dma_start(bias_sb[:], bias[:])

            # Single-partition output staging: compute engines may only
            # address APs with a 32-aligned base partition, so out rows
            # live along the free axis at partition 0, drained in halves.
            HALF = B // 2
            out_sb = opool.tile([1, HALF * S], f32)

            for b in range(B):
                # One 4 MiB load per batch minimizes per-trigger overhead.
                # DVE can only start a chunk's mul after the whole chunk
                # lands, so whole-batch chunks leave DVE a full 8.6 us mul
                # behind the stream at the end; the last two batches load
                # in halves then eighths so the post-stream drain is one
                # small chunk's chain, not a batch's.
                xb = xpool.tile([P, F], f32, tag="xb")
                prod = ppool.tile([P, F], bf16, tag="prod")
                if b == B - 1:
                    chunks = [2 * S] * 7 + [S] * 2
                elif b >= B - 3:
                    chunks = [4 * S] * 4
                else:
                    chunks = [F]
                ps = pspool.tile([1, S], f32)
                # bias fold-in: K=1 fp32 matmul opens the accumulation
                # group (512 cols, exact, negligible PE time)
                nc.tensor.matmul(
                    ps[:], ones_f32[0:1, 0:1], bias_sb[:], start=True, stop=False
                )
                r0 = 0
                for h, ch in enumerate(chunks):
                    r1 = r0 + ch
                    nc.sync.dma_start(xb[:, r0:r1], x[b, :, r0:r1])
                    nc.vector.tensor_mul(
                        prod[:, r0:r1], xb[:, r0:r1], w_sb[:, r0:r1]
                    )
                    for cb in range(r0 // S, r1 // S):
                        rhs = prod[:, cb * S : (cb + 1) * S]
                        last = cb == CB - 1
                        nc.tensor.matmul(
                            ps[:], ones[:], rhs, start=False, stop=last
                        )
                    r0 = r1

                nc.scalar.activation(
                    out_sb[0:1, (b % HALF) * S : (b % HALF + 1) * S],
                    ps[:],
                    mybir.ActivationFunctionType.Relu,
                )
                if b == HALF - 1:
                    # Scalar ring: on the sync ring this drain's wait-on-ACT
                    # would block later x triggers (FIFO per engine).
                    nc.scalar.dma_start(
                        out[0:HALF].unsqueeze(0),
                        out_sb[:].rearrange("p (b s) -> p b s", b=HALF),
                    )
                if b == B - 2:
                    # Rows 16..30 drain mid-stream too, so the post-ACT
                    # serial tail only carries row 31's 2 KiB.
                    nc.scalar.dma_start(
                        out[HALF : B - 1].unsqueeze(0),
                        out_sb[:, 0 : (HALF - 1) * S].rearrange(
                            "p (b s) -> p b s", b=HALF - 1
                        ),
                    )

            # Scalar ring: same engine as the final relu, so the trigger
            # issues without a cross-engine semaphore hop.
            nc.scalar.dma_start(
                out[B - 1 :].unsqueeze(0),
                out_sb[:, (HALF - 1) * S :].rearrange("p (b s) -> p b s", b=1),
            )

    nc.compile()
    return nc


def _get_nc():
    if "nc" not in _nc_cache:
        _nc_cache["nc"] = _build()
    return _nc_cache["nc"]


def _shard_inputs(x, weights, bias):
    x = np.asarray(x)
    weights = np.asarray(weights)
    bias = np.asarray(bias)
    in_maps = []
    for i in range(N_CORES):
        sl = slice(i * S, (i + 1) * S)
        # The [B, 2048, S] shard reshaped to [B, 128, 8192] realizes the
        # p-outer layout: (c, s) -> (c // 16, (c % 16) * 512 + s).
        in_maps.append(
            {
                "xs": np.ascontiguousarray(
                    x[:, :, sl], dtype=np.float32
                ).reshape(B, P, F),
                "ws": np.ascontiguousarray(
                    weights[:, sl], dtype=np.float32
                ).reshape(P, F),
                "bs": np.ascontiguousarray(
                    bias[sl].reshape(1, S), dtype=np.float32
                ),
            }
        )
    return in_maps


def _run(inputs, trace=False, trace_cores=None):
    from concourse import bass_utils

    nc = _get_nc()
    in_maps = _shard_inputs(inputs["x"], inputs["weights"], inputs["bias"])
    res = bass_utils.run_bass_kernel_spmd(
        nc,
        in_maps,
        core_ids=list(range(N_CORES)),
        trace=trace,
        trace_cores=trace_cores,
    )
    out = np.concatenate([r["out"] for r in res.results], axis=1)
    return out, res


def kernel(x, weights, bias):
    out, _ = _run({"x": x, "weights": weights, "bias": bias})
    return out


# revision 8
# speedup vs baseline: 1.2261x; 1.0247x over previous
"""Trainium2 Bass kernel for: out = relu(einsum('bcs,cs->bs', x, w) + bias).

Full shapes: x [32, 2048, 4096] f32, w [2048, 4096] f32, bias [4096] f32.
Sharding: the s-axis (4096) is split across 8 cores (512 each). Each core
reads its x slice (128 MiB) and w/bias slice (4 MiB) once — the minimum
possible HBM traffic — and produces out[:, s_slice]. Gather = concat.

Per-core dataflow (partitions = 16-channel group, free = (cb, s)):
  DMA   x[b] slice  -> SBUF [128, 8192]             (4 MiB per batch)
  DVE   prod = xb * w   (fp32 mul, bf16 output tile)
  PE    ones-matmul per 512-col block, accumulating the 128-partition
        reduction of each [128, 512] product block into PSUM [1, 512];
        the bias row is folded in as a K=1 fp32 matmul that opens the
        accumulation group.
  ACT   relu during PSUM -> SBUF copy into out row b
  DMA   out [32, 512] -> DRAM

Layout: channels are mapped p-outer — partition p holds channels
[16p, 16p+16) — so each partition's DMA line is 32 KiB contiguous in
DRAM (the cb-outer layout gives 16x 2 KiB strided lines, which measured
345 GB/s busy-rate vs the 358 GB/s HBM wire). The channel permutation
is applied consistently to x and w, and the reduction is permutation-
invariant, so the result is unchanged.

Precision: the x*w products are rounded to bf16 before the PE reduction
(the reduction itself accumulates in fp32 PSUM). bf16 moving operands
stream at 1 col/cycle vs 4 for fp32, which keeps PE (~3.5 us/batch) and
DVE (~8.6 us/batch, no fold offload needed) both well under the 11.8 us
DMA period — so the end-of-stream drain is just the last chunk's chain
instead of a ~25 us engine backlog. Product rounding error accumulates
as ~2^-9 relative l2 on the output (gate is 2e-2).
"""

import numpy as np

B, C, S_FULL = 32, 2048, 4096
N_CORES = 8
S = S_FULL // N_CORES          # 512 s-values per core
P = 128                        # SBUF partitions
CB = C // P                    # 16 channel blocks (512-col matmul blocks)
F = CB * S                     # 8192 free elems per partition

_nc_cache = {}


def _build():
    import concourse.bacc as bacc
    import concourse.mybir as mybir
    import concourse.tile as tile

    f32 = mybir.dt.float32
    bf16 = mybir.dt.bfloat16
    nc = bacc.Bacc(
        "TRN2",
        target_bir_lowering=False,
        debug=False,
        enable_asserts=False,
        num_devices=N_CORES,
    )

    # p-outer channel layout: x[b, p, q*S + s] = x_orig[b, 16p + q, s].
    # Each partition's per-batch line is one 32 KiB contiguous DRAM run.
    x = nc.dram_tensor("xs", [B, P, F], f32, kind="ExternalInput").ap()
    w = nc.dram_tensor("ws", [P, F], f32, kind="ExternalInput").ap()
    bias = nc.dram_tensor("bs", [1, S], f32, kind="ExternalInput").ap()
    out = nc.dram_tensor("out", [B, S], f32, kind="ExternalOutput").ap()

    with tile.TileContext(nc) as tc:
        with (
            tc.tile_pool(name="const", bufs=1) as cpool,
            tc.tile_pool(name="xp", bufs=4) as xpool,
            tc.tile_pool(name="ps", bufs=4, space="PSUM") as pspool,
            tc.tile_pool(name="op", bufs=1) as opool,
        ):
            # w leads the Sync ring ahead of the x stream: a second
            # concurrent HWDGE stream measures ~8% slower per packet,
            # which costs more than the serial weight load.
            w_sb = cpool.tile([P, F], f32)
            nc.sync.dma_start(w_sb[:], w[:])
            # bf16 copy for the muls (one-time, overlaps the x0 load)
            w_bf = cpool.tile([P, F], bf16)
            nc.vector.tensor_copy(w_bf[:], w_sb[:])

            # lhsT of the reduction matmuls, bf16 to match the product
            # tiles (memset can't write bf16; round via DVE copy).
            ones_f32 = cpool.tile([P, 1], f32)
            nc.vector.memset(ones_f32[:], 1.0)
            ones = cpool.tile([P, 1], bf16)
            nc.vector.tensor_copy(ones[:], ones_f32[:])

            # scalar ring: keeps this 2 KiB transfer (and its trigger) out
            # of the w -> x0 handoff on the sync ring
            bias_sb = cpool.tile([1, S], f32)
            nc.scalar.dma_start(bias_sb[:], bias[:])

            # Single-partition output staging: compute engines may only
            # address APs with a 32-aligned base partition, so out rows
            # live along the free axis at partition 0, drained in halves.
            HALF = B // 2
            out_sb = opool.tile([1, HALF * S], f32)

            for b in range(B):
                # One 4 MiB load per batch minimizes per-trigger overhead.
                # DVE can only start a chunk's mul after the whole chunk
                # lands, so whole-batch chunks leave DVE a full 8.6 us mul
                # behind the stream at the end; the last two batches load
                # in halves then eighths so the post-stream drain is one
                # small chunk's chain, not a batch's.
                # x lands as bf16 via SWDGE cast-DMA: halves the SBUF
                # write traffic (the 435 GB/s fabric side binds while
                # this core is alone on its HBM stack), and the all-bf16
                # in-place mul runs in DVE 2x perf mode.
                xb = xpool.tile([P, F], bf16, tag="xb")
                if b == B - 1:
                    chunks = [2 * S] * 7 + [S] * 2
                elif b >= B - 3:
                    chunks = [4 * S] * 4
                else:
                    chunks = [F]
                ps = pspool.tile([1, S], f32)
                # bias fold-in: K=1 fp32 matmul opens the accumulation
                # group (512 cols, exact, negligible PE time)
                nc.tensor.matmul(
                    ps[:], ones_f32[0:1, 0:1], bias_sb[:], start=True, stop=False
                )
                r0 = 0
                for h, ch in enumerate(chunks):
                    r1 = r0 + ch
                    nc.gpsimd.dma_start(xb[:, r0:r1], x[b, :, r0:r1])
                    nc.vector.tensor_mul(
                        xb[:, r0:r1], xb[:, r0:r1], w_bf[:, r0:r1]
                    )
                    for cb in range(r0 // S, r1 // S):
                        rhs = xb[:, cb * S : (cb + 1) * S]
                        last = cb == CB - 1
                        nc.tensor.matmul(
                            ps[:], ones[:], rhs, start=False, stop=last
                        )
                    r0 = r1

                nc.scalar.activation(
                    out_sb[0:1, (b % HALF) * S : (b % HALF + 1) * S],
                    ps[:],
                    mybir.ActivationFunctionType.Relu,
                )
                if b == HALF - 1:
                    # Scalar ring: on the sync ring this drain's wait-on-ACT
                    # would block later x triggers (FIFO per engine).
                    nc.scalar.dma_start(
                        out[0:HALF].unsqueeze(0),
                        out_sb[:].rearrange("p (b s) -> p b s", b=HALF),
                    )
                if b == B - 2:
                    # Rows 16..30 drain mid-stream too, so the post-ACT
                    # serial tail only carries row 31's 2 KiB.
                    nc.scalar.dma_start(
                        out[HALF : B - 1].unsqueeze(0),
                        out_sb[:, 0 : (HALF - 1) * S].rearrange(
                            "p (b s) -> p b s", b=HALF - 1
                        ),
                    )

            # Scalar ring: same engine as the final relu, so the trigger
            # issues without a cross-engine semaphore hop.
            nc.scalar.dma_start(
                out[B - 1 :].unsqueeze(0),
                out_sb[:, (HALF - 1) * S :].rearrange("p (b s) -> p b s", b=1),
            )

    nc.compile()
    return nc


def _get_nc():
    if "nc" not in _nc_cache:
        _nc_cache["nc"] = _build()
    return _nc_cache["nc"]


def _shard_inputs(x, weights, bias):
    x = np.asarray(x)
    weights = np.asarray(weights)
    bias = np.asarray(bias)
    in_maps = []
    for i in range(N_CORES):
        sl = slice(i * S, (i + 1) * S)
        # The [B, 2048, S] shard reshaped to [B, 128, 8192] realizes the
        # p-outer layout: (c, s) -> (c // 16, (c % 16) * 512 + s).
        in_maps.append(
            {
                "xs": np.ascontiguousarray(
                    x[:, :, sl], dtype=np.float32
                ).reshape(B, P, F),
                "ws": np.ascontiguousarray(
                    weights[:, sl], dtype=np.float32
                ).reshape(P, F),
                "bs": np.ascontiguousarray(
                    bias[sl].reshape(1, S), dtype=np.float32
                ),
            }
        )
    return in_maps


def _run(inputs, trace=False, trace_cores=None):
    from concourse import bass_utils

    nc = _get_nc()
    in_maps = _shard_inputs(inputs["x"], inputs["weights"], inputs["bias"])
    res = bass_utils.run_bass_kernel_spmd(
        nc,
        in_maps,
        core_ids=list(range(N_CORES)),
        trace=trace,
        trace_cores=trace_cores,
    )
    out = np.concatenate([r["out"] for r in res.results], axis=1)
    return out, res


def kernel(x, weights, bias):
    out, _ = _run({"x": x, "weights": weights, "bias": bias})
    return out
